# revision 5
# baseline (speedup 1.0000x reference)
"""Bass/Tile TRN2 kernel for nn_BasicRNN: out = scan(tanh(x@Wx + h@Wh) + h) @ Wout.

Data-parallel over batch across 8 NeuronCores (32 rows/core), recurrence
sequential in time on each core. No collectives; host gathers shards.

Numerics (validated in numpy emulation): the recurrence amplifies per-step
error ~80x, so every matmul operand is split into fp16 hi+lo pairs
(error ~2^-22): u@Wh uses u_hi|u_lo vs Wh_hi plus u_hi vs Wh_lo, xproj
splits both x and Wx 3-term. The output projection is final (no
amplification) so a single fp16 pass suffices (3.2e-4 total).

Structure per step (increment form, M_t = M_{t-1} + u_{t-1}@Wh in a
persistent single-bank PSUM accumulator [128, (m,b)]):
  - lo-pass: 16 MMs Wh_lo[k,m] x u_hi[k] (N=32)
  - hi-pass: 16 MMs Wh_hi[k,m] x [u_hi|u_lo][k] (N=64) with a 0-stride
    broadcast OUTPUT AP so both halves accumulate into the same 32
    columns (verified on HW) -> macc holds the full increment, one bank.
  - pointwise in two halves (m-chunks {0,1} / {2,3}) pipelined under the
    PE stream: P = macc_half + xp_half (DVE), tanh fp32 (ACT),
    u_hi = cast (DVE), u_lo = sub (DVE); h += u and hist cast are
    off-chain (DVE tail / ACT).
  - whmm of step t+1 is emitted loA,hiA,loB,hiB so loA only waits on
    castA(t) and the chain of half B hides under the A matmuls.
x hi/lo casts are hoisted to the prologue (x is loop-invariant), xproj
group banks are cleared via start=True on the first MM per bank, and
xproj/outproj matmuls are chopped into small thunks drained after each
step's whmm as PE filler during the pointwise tail.
"""

import sys

sys.path.insert(0, "/opt/trn_rl_repo")

from collections import deque

import numpy as np

import concourse.bass as bass  # noqa: F401
import concourse.tile as tile
from concourse import bacc, mybir
from concourse.bass_utils import run_bass_kernel_spmd

FP = mybir.dt.float32
F16 = mybir.dt.float16
TANH = mybir.ActivationFunctionType.Tanh
COPY = mybir.ActivationFunctionType.Copy

B, D, T, H, OUT = 256, 256, 256, 512, 256
NCORES = 8
BC = B // NCORES  # 32 batch rows per core
P = 128
DC = D // P  # 2 d-chunks
HC = H // P  # 4 h-chunks


def build(T_=T, G=8, reps=1, parts=("whmm", "pointwise", "xproj", "outproj"),
          hist_on_act=True):
    parts = set(parts)
    NG = T_ // G
    GB = G * BC            # (t, b) free width of one group = 256
    TPM = P // BC          # timesteps per outproj M-chunk = 4
    MCG = GB // P          # outproj M-chunks per group = 2
    UW = HC * 2 * BC       # u2 width (k, hi|lo, b) = 256
    HB = HC * BC           # packed h width (c, b) = 128
    HHB = HB // 2          # half of it = 64
    assert T_ % G == 0 and GB <= 512

    nc = bacc.Bacc("TRN2", target_bir_lowering=False, debug=False, num_devices=NCORES)

    x_d = nc.declare_dram_parameter("x", [BC, D, T_], FP, isOutput=False)
    wx_d = nc.declare_dram_parameter("Wx", [D, H], FP, isOutput=False)
    wh_d = nc.declare_dram_parameter("Wh", [H, H], FP, isOutput=False)
    b_d = nc.declare_dram_parameter("b", [H], FP, isOutput=False)
    wo_d = nc.declare_dram_parameter("Wout", [H, OUT], FP, isOutput=False)
    bo_d = nc.declare_dram_parameter("bout", [OUT], FP, isOutput=False)
    init_d = nc.declare_dram_parameter("init_state", [1, H], FP, isOutput=False)
    out_d = nc.declare_dram_parameter("out", [BC, T_, OUT], FP, isOutput=True)

    with tile.TileContext(nc) as tc:
        with (
            tc.tile_pool(name="const", bufs=1) as const,
            tc.tile_pool(name="xbuf", bufs=1) as xbuf,
            tc.tile_pool(name="h0p", bufs=1) as h0p,
            tc.tile_pool(name="hist", bufs=3) as hist_pool,
            tc.tile_pool(name="upool", bufs=3) as upool,
            tc.tile_pool(name="ufpool", bufs=3) as ufpool,
            tc.tile_pool(name="hfpool", bufs=3) as hfpool,
            tc.tile_pool(name="ppool", bufs=3) as ppool,
            tc.tile_pool(name="xps", bufs=2) as xps_pool,
            tc.tile_pool(name="stg", bufs=4) as stg_pool,
            tc.tile_pool(name="xpp", bufs=2, space="PSUM") as xp_psum,
            tc.tile_pool(name="macc", bufs=1, space="PSUM") as macc_pool,
            tc.tile_pool(name="opp", bufs=2, space="PSUM") as op_psum,
        ):
            # ---------------- one-time prologue: weights + x ----------------
            def load_split(dram_ap, rows, cols, nm):
                f = const.tile([rows, cols], FP, name=f"{nm}f")
                hi = const.tile([rows, cols], F16, name=f"{nm}h")
                lo = const.tile([rows, cols], F16, name=f"{nm}l")
                nc.sync.dma_start(out=f[:, :], in_=dram_ap)
                nc.vector.tensor_copy(hi[:, :], f[:, :])
                nc.vector.tensor_sub(lo[:, :], f[:, :], hi[:, :])
                return hi, lo

            wh_hi, wh_lo = [], []
            for k in range(HC):
                hi, lo = load_split(wh_d[k * P:(k + 1) * P, :], P, H, f"wh{k}")
                wh_hi.append(hi); wh_lo.append(lo)
            wx_hi, wx_lo = [], []
            for d in range(DC):
                hi, lo = load_split(wx_d[d * P:(d + 1) * P, :], P, H, f"wx{d}")
                wx_hi.append(hi); wx_lo.append(lo)
            wo_hi = []
            for k in range(HC):
                f = const.tile([P, OUT], FP, name=f"wo{k}f")
                hi = const.tile([P, OUT], F16, name=f"wo{k}h")
                nc.sync.dma_start(out=f[:, :], in_=wo_d[k * P:(k + 1) * P, :])
                nc.vector.tensor_copy(hi[:, :], f[:, :])
                wo_hi.append(hi)
            b_hi, b_lo = load_split(b_d[:].rearrange("(o h) -> o h", o=1), 1, H, "b")
            bo_hi, bo_lo = load_split(bo_d[:].rearrange("(o h) -> o h", o=1), 1, OUT, "bo")

            ones = const.tile([1, 512], F16, name="ones")
            nc.vector.memset(ones[:, :], 1.0)
            zrow = const.tile([1, P], F16, name="zrow")
            nc.vector.memset(zrow[:, :], 0.0)

            init_sb = const.tile([P, HC], FP, name="initsb")
            nc.sync.dma_start(
                out=init_sb[:, :], in_=init_d[0, :].rearrange("(c p) -> p c", p=P)
            )

            # x resident in SBUF: fp32 staged in, then hi/lo fp16 for all T
            # (loop-invariant, so cast once here, not per group).
            x_f = [xbuf.tile([P, BC * T_], FP, name=f"xf{d}") for d in range(DC)]
            x_hi = [xbuf.tile([P, BC * T_], F16, name=f"xh{d}") for d in range(DC)]
            x_lo = [xbuf.tile([P, BC * T_], F16, name=f"xl{d}") for d in range(DC)]
            NQ = 4 if T_ % 4 == 0 else 1
            TQ = T_ // NQ
            for q in range(NQ):
                for d in range(DC):
                    dst = x_f[d].rearrange("p (b t) -> p b t", b=BC)[:, :, q * TQ:(q + 1) * TQ]
                    src = x_d[:, d * P:(d + 1) * P, q * TQ:(q + 1) * TQ].rearrange("b d t -> d b t")
                    nc.sync.dma_start(out=dst, in_=src)
            for q in range(NQ):
                for d in range(DC):
                    sl = slice(q * (BC * TQ), (q + 1) * (BC * TQ))
                    # note: x_f free layout is (b, t); hi/lo keep the same
                    nc.vector.tensor_copy(x_hi[d][:, sl], x_f[d][:, sl])
                    nc.vector.tensor_sub(x_lo[d][:, sl], x_f[d][:, sl], x_hi[d][:, sl])

            # ---------------- per-run body (repeatable for timing) ----------
            def body():
                h0_f = h0p.tile([P, HB], FP, name="h0f")
                nc.vector.memset(h0_f[:, :], 0.0)
                for c in range(HC):
                    nc.vector.tensor_scalar_add(
                        h0_f[:, c * BC:(c + 1) * BC],
                        h0_f[:, c * BC:(c + 1) * BC],
                        init_sb[:, c:c + 1],
                    )
                u2_0 = h0p.tile([P, UW], F16, name="u20")
                u2_0v = u2_0.rearrange("p (k two b) -> p k two b", k=HC, two=2)
                h0_3 = h0_f.rearrange("p (c b) -> p c b", c=HC)
                nc.vector.tensor_copy(u2_0v[:, :, 0, :], h0_3)
                nc.vector.tensor_sub(u2_0v[:, :, 1, :], h0_3, u2_0v[:, :, 0, :])
                # persistent M accumulator, one bank per half so the DVE
                # read of half A never shares a bank with concurrent PE
                # writes to half B (PE-W + DVE-R same bank is fatal)
                maccs = [macc_pool.tile([P, HHB], FP, name=f"macc{i}") for i in range(2)]
                for m_ in maccs:
                    nc.tensor.matmul(
                        out=m_[:, :], lhsT=zrow[0:1, :], rhs=ones[0:1, 0:HHB],
                        start=True, stop=False, skip_group_check=True,
                    )

                xp_tiles = {}
                xps_tiles = {}
                hist_tiles = {}
                fillers = deque()

                def xproj_thunks(g):
                    """xp[g] psum [128, (m, t, b)]; 3-term hi/lo; start=True
                    clears each bank via the first MM touching it."""
                    xp = xp_psum.tile([P, HC * GB], FP, name=f"xp{g}", tag="xp")
                    xp_tiles[g] = xp
                    ths = []
                    if "xproj" not in parts:
                        return ths
                    BANK_F32 = 512
                    bank_started = set()

                    def mk(m, terms, g=g, xp=xp):
                        def th(m=m, terms=terms, xp=xp):
                            out_ap = xp[:, m * GB:(m + 1) * GB]
                            bank = (m * GB) // BANK_F32
                            for lhsT, rhs_tile in terms:
                                first = bank not in bank_started
                                bank_started.add(bank)
                                rhs = rhs_tile.rearrange("p (b t) -> p t b", b=BC)[
                                    :, g * G:(g + 1) * G, :]
                                nc.tensor.matmul(
                                    out=out_ap,
                                    lhsT=lhsT[:, m * P:(m + 1) * P],
                                    rhs=rhs,
                                    start=first, stop=False, skip_group_check=True,
                                )
                        return th

                    for m in range(HC):
                        ths.append(mk(m, [(wx_hi[0], x_hi[0]), (wx_hi[0], x_lo[0])]))
                        ths.append(mk(m, [(wx_lo[0], x_hi[0]), (wx_hi[1], x_hi[1])]))
                        ths.append(mk(m, [(wx_hi[1], x_lo[1]), (wx_lo[1], x_hi[1])]))

                    def bias_th(xp=xp, g=g):
                        for m in range(HC):
                            out_ap = xp[:, m * GB:(m + 1) * GB]
                            for brow in (b_hi, b_lo):
                                nc.tensor.matmul(
                                    out=out_ap,
                                    lhsT=brow[0:1, m * P:(m + 1) * P],
                                    rhs=ones[0:1, 0:GB],
                                    start=False, stop=False, skip_group_check=True,
                                )
                    ths.append(bias_th)

                    # stage to SBUF re-laid-out as (t, m, b) so per-step
                    # half slices are contiguous 64-col runs
                    xps = xps_pool.tile([P, HC * GB], FP, name=f"xps{g}", tag="xps")
                    xps_tiles[g] = xps
                    xp4 = xp.rearrange("p (m t b) -> p m t b", m=HC, t=G)
                    xps4 = xps.rearrange("p (t m b) -> p t m b", t=G, m=HC)

                    def stage(lo, hi, xp4=xp4, xps4=xps4):
                        nc.vector.tensor_copy(
                            xps4[:, lo:hi, :, :],
                            xp4[:, :, lo:hi, :].rearrange("p m t b -> p t m b"),
                        )
                    ths.append(lambda st=stage: st(0, G // 2))
                    ths.append(lambda st=stage: st(G // 2, G))
                    return ths

                def outproj_thunks(g):
                    hist = hist_tiles[g]
                    ths = []
                    for mc in range(MCG):
                        ops = op_psum.tile([P, OUT], FP, name=f"op{g}_{mc}", tag="op")

                        def mm_half(ks, first, mc=mc, hist=hist, ops=ops):
                            fst = first
                            for k in ks:
                                lhsT = hist[:, k * G * BC + mc * P: k * G * BC + (mc + 1) * P]
                                nc.tensor.matmul(
                                    out=ops[:, :], lhsT=lhsT, rhs=wo_hi[k][:, :],
                                    start=fst, stop=False,
                                )
                                fst = False

                        def tail(mc=mc, g=g, ops=ops):
                            for brow in (bo_hi, bo_lo):
                                nc.tensor.matmul(
                                    out=ops[:, :], lhsT=ones[0:1, 0:P], rhs=brow[0:1, :],
                                    start=False, stop=False,
                                )
                            nc.tensor.matmul(
                                out=ops[:, :], lhsT=zrow[0:1, 0:P], rhs=ones[0:1, 0:OUT],
                                start=False, stop=True,
                            )
                            stg = stg_pool.tile([P, OUT], FP, name=f"st{g}_{mc}", tag="stg")
                            nc.vector.tensor_copy(stg[:, :], ops[:, :])
                            t0 = g * G + mc * TPM
                            dst = out_d[:, t0:t0 + TPM, :].rearrange("b t o -> t b o")
                            nc.sync.dma_start(out=dst, in_=stg[:, :])

                        ths.append(lambda mm=mm_half: mm((0, 1), True))
                        ths.append(lambda mm=mm_half: mm((2, 3), False))
                        ths.append(tail)
                    return ths

                for th in xproj_thunks(0):
                    th()

                prev_f = h0_f[:, :]     # h_{t-1} fp32 [128, (c, b)]
                prev_u = u2_0           # u_{t-1} fp16 [128, (k, 2, b)]

                halves = ((0, 2), (2, 4))  # m-chunk ranges A, B

                for t in range(T_):
                    g, tl = divmod(t, G)
                    if tl == 0:
                        while fillers:
                            fillers.popleft()()
                        hist_tiles[g] = hist_pool.tile(
                            [P, G * HB], F16, name=f"hist{g}", tag="hist"
                        )
                        if g + 1 < NG:
                            fillers.extend(xproj_thunks(g + 1))
                        if g >= 1 and "outproj" in parts:
                            fillers.extend(outproj_thunks(g - 1))

                    pu = prev_u.rearrange("p (k two b) -> p k two b", k=HC, two=2)

                    # ---- whmm: loA, hiA, loB, hiB; all accumulate into the
                    # single-group macc (hi via broadcast-out) ----
                    if "whmm" in parts:
                        for hi_, (m0, m1) in enumerate(halves):
                            macc = maccs[hi_]
                            for m in range(m0, m1):
                                out32 = macc[:, (m - m0) * BC:(m - m0 + 1) * BC]
                                for k in range(HC):
                                    nc.tensor.matmul(
                                        out=out32,
                                        lhsT=wh_lo[k][:, m * P:(m + 1) * P],
                                        rhs=pu[:, k, 0, :],
                                        start=False, stop=False, skip_group_check=True,
                                    )
                            for m in range(m0, m1):
                                outb = macc[:, (m - m0) * BC:(m - m0 + 1) * BC].rearrange(
                                    "p (o b) -> p o b", o=1).broadcast_to([P, 2, BC])
                                for k in range(HC):
                                    nc.tensor.matmul(
                                        out=outb,
                                        lhsT=wh_hi[k][:, m * P:(m + 1) * P],
                                        rhs=pu[:, k, :, :],
                                        start=False, stop=False, skip_group_check=True,
                                    )

                    if "pointwise" not in parts:
                        for _ in range(6):
                            if fillers:
                                fillers.popleft()()
                        continue

                    xps = xps_tiles[g]
                    xps3 = xps.rearrange("p (t mb) -> p t mb", t=G)

                    u2 = upool.tile([P, UW], F16, name=f"u{t}", tag="u")
                    u2v = u2.rearrange("p (k two b) -> p k two b", k=HC, two=2)
                    uf = ufpool.tile([P, HB], FP, name=f"uf{t}", tag="uf")
                    pt = ppool.tile([P, HB], FP, name=f"p{t}", tag="p")
                    hf = hfpool.tile([P, HB], FP, name=f"hf{t}", tag="hf")

                    for hi_, (m0, m1) in enumerate(halves):
                        lo_c, hi_c = m0 * BC, m1 * BC
                        # P = macc_half + xp_half   (both contiguous 64 cols)
                        nc.vector.tensor_add(
                            pt[:, lo_c:hi_c],
                            maccs[hi_][:, :],
                            xps3[:, tl, lo_c:hi_c],
                        )
                        # tanh fp32
                        nc.scalar.activation(uf[:, lo_c:hi_c], pt[:, lo_c:hi_c], TANH)
                        # u_hi = fp16 cast ; u_lo = uf - u_hi
                        ufh = uf.rearrange("p (c b) -> p c b", c=HC)[:, m0:m1, :]
                        nc.vector.tensor_copy(u2v[:, m0:m1, 0, :], ufh)
                        nc.vector.tensor_sub(u2v[:, m0:m1, 1, :], ufh, u2v[:, m0:m1, 0, :])

                    # off-chain tail: h accumulate + hist cast
                    nc.vector.tensor_add(hf[:, :], uf[:, :], prev_f)
                    hdst = hist_tiles[g].rearrange(
                        "p (c t b) -> p c t b", c=HC, t=G)[:, :, tl, :]
                    hsrc = hf.rearrange("p (c b) -> p c b", c=HC)
                    if hist_on_act:
                        nc.scalar.activation(hdst, hsrc, COPY)
                    else:
                        nc.vector.tensor_copy(hdst, hsrc)

                    prev_f = hf[:, :]
                    prev_u = u2

                    for _ in range(6):
                        if fillers:
                            fillers.popleft()()

                while fillers:
                    fillers.popleft()()
                if "outproj" in parts:
                    for th in outproj_thunks(NG - 1):
                        th()

            if reps > 4:
                with tc.For_i(0, reps, 1):
                    body()
            else:
                for _ in range(reps):
                    body()

    nc.compile()
    return nc


_NC_CACHE = {}


def _get_nc(T_=T, G=8, reps=1):
    key = (T_, G, reps)
    if key not in _NC_CACHE:
        _NC_CACHE[key] = build(T_, G, reps)
    return _NC_CACHE[key]


def run(inputs, T_=T, G=8, reps=1):
    nc = _get_nc(T_, G, reps)
    x = np.ascontiguousarray(np.asarray(inputs["x"], dtype=np.float32))
    shared = {
        k: np.ascontiguousarray(np.asarray(inputs[k], dtype=np.float32))
        for k in ("Wx", "Wh", "b", "Wout", "bout", "init_state")
    }
    core_ids = list(range(NCORES))
    in_maps = [{"x": x[c * BC:(c + 1) * BC], **shared} for c in core_ids]
    res = run_bass_kernel_spmd(nc, in_maps, core_ids)
    out = np.concatenate([res.results[c]["out"] for c in core_ids], axis=0)
    return out


def kernel(**inputs):
    return run(inputs)


if __name__ == "__main__":
    import time

    t0 = time.time()
    _get_nc()
    print(f"build: {time.time() - t0:.1f}s")
